# revision 9
# baseline (speedup 1.0000x reference)
"""Trainium2 Bass kernel for nn_ExpressionModule_2267742732789.

Expression tree (DEPTH=4, preorder params, 25 scalars), elementwise over x:
    x2 = x*x
    t1 = tanh(p7 *x2)   t2 = tanh(p8 *x2)   u1 = p4*t1 + p5*t2 + p6
    t3 = tanh(p12*x2)   t4 = tanh(p13*x2)   u2 = p9*t3 + p10*t4 + p11
    v1 = tanh(p3 * u1*u2)
    t5 = tanh(p18*x2)   t6 = tanh(p19*x2)   u3 = p15*t5 + p16*t6 + p17
    t7 = tanh(p23*x2)   t8 = tanh(p24*x2)   u4 = p20*t7 + p21*t8 + p22
    v2 = tanh(p14 * u3*u4)
    out = p0*v1 + p1*v2 + p2

Sharding: x (16M fp32) split evenly across the 8 NeuronCores (data
parallel, per the elementwise structure); the 25 scalar params are baked
into instruction immediates at call time (JIT specialization -- the kernel
recompiles for new param values, so it is correct for any input).

Engine split per 2M-element core shard (8 chunks of [128, 2048] fp32),
chosen by hardware A/B benchmarking (slope timing of K-pass kernels):
  ACT  (11 passes/elem): x^2 via Square + 8 leaf tanh (scale=p_k folded
        into the free pre-affine) + 2 mid tanh          ~158 us busy
  DVE  (7 ops/chunk): 4 waff combines (scalar_tensor_tensor),
        2 tree products (tensor_tensor), final combine   ~158 us busy
  GPSIMD (5 ops/chunk): affine terms t*w + b (tensor_scalar)
  Measured steady-state on TRN2: ~158 us/core; DMA (16.8 MB @ ~358 GB/s
  = 47 us) fully overlapped.

Findings that shaped this (from HW A/B runs):
  - DVE ops carry a ~0.5 us/op drain penalty on HW that the cost model
    misses -> minimizing DVE op count matters more than cycle balance.
  - Pool (GPSIMD) 2-input ops (tensor_tensor/STT) are ~2.6x slower than
    DVE and poison the critical path; only 1-input tensor_scalar is used.
  - ACT had slack (an extra full pass was free), so x^2 moved there.
"""

import os
import sys

import numpy as np

sys.path.insert(0, "/opt/trn_rl_repo")

import concourse.bacc as bacc
import concourse.mybir as mybir
from concourse import tile
from concourse.bass_utils import run_bass_kernel_spmd

N = 16777216
NCORES = 8
E = N // NCORES  # 2_097_152 per core
P = 128
COLS = E // P  # 16384 per-lane elements
FD = 2048
NCHUNK = COLS // FD  # 8

F32 = mybir.dt.float32
MULT = mybir.AluOpType.mult
ADD = mybir.AluOpType.add
TANH = mybir.ActivationFunctionType.Tanh
SQUARE = mybir.ActivationFunctionType.Square


def build_nc(p, passes=1):
    """Build the SPMD Bass program with params p (25 floats) baked in.

    passes>1 repeats the computation (same in/out) for benchmarking.
    """
    nc = bacc.Bacc("TRN2", target_bir_lowering=False, debug=False)
    x_h = nc.dram_tensor("x", [P, COLS], F32, kind="ExternalInput")
    o_h = nc.dram_tensor("out", [P, COLS], F32, kind="ExternalOutput")

    with tile.TileContext(nc) as tc:
        with (
            tc.tile_pool(name="px", bufs=3) as px,
            tc.tile_pool(name="po", bufs=3) as po,
            tc.tile_pool(name="px2", bufs=3) as px2,
            tc.tile_pool(name="pt", bufs=5) as pt,
            tc.tile_pool(name="pa", bufs=3) as pa,
            tc.tile_pool(name="pu", bufs=3) as pu,
            tc.tile_pool(name="pm", bufs=2) as pm,
            tc.tile_pool(name="pv", bufs=2) as pv,
        ):
            for c in [c for _ in range(passes) for c in range(NCHUNK)]:
                sl = slice(c * FD, (c + 1) * FD)
                xt = px.tile([P, FD], F32, tag="x")
                nc.sync.dma_start(out=xt[:], in_=x_h[:, sl])
                x2 = px2.tile([P, FD], F32, tag="x2")
                nc.scalar.activation(x2[:], xt[:], SQUARE)

                def waff(s_a, s_b, w0, w1, b0):
                    ta = pt.tile([P, FD], F32, tag="t")
                    nc.scalar.activation(ta[:], x2[:], TANH, scale=s_a)
                    tb = pt.tile([P, FD], F32, tag="t")
                    nc.scalar.activation(tb[:], x2[:], TANH, scale=s_b)
                    aa = pa.tile([P, FD], F32, tag="a")
                    nc.gpsimd.tensor_scalar(aa[:], ta[:], w0, b0, MULT, ADD)
                    uu = pu.tile([P, FD], F32, tag="u")
                    nc.vector.scalar_tensor_tensor(uu[:], tb[:], w1, aa[:], MULT, ADD)
                    return uu

                u1 = waff(p[7], p[8], p[4], p[5], p[6])
                u2 = waff(p[12], p[13], p[9], p[10], p[11])
                m1 = pm.tile([P, FD], F32, tag="m")
                nc.vector.tensor_tensor(m1[:], u1[:], u2[:], MULT)
                v1 = pv.tile([P, FD], F32, tag="v")
                nc.scalar.activation(v1[:], m1[:], TANH, scale=p[3])

                u3 = waff(p[18], p[19], p[15], p[16], p[17])
                u4 = waff(p[23], p[24], p[20], p[21], p[22])
                m2 = pm.tile([P, FD], F32, tag="m")
                nc.vector.tensor_tensor(m2[:], u3[:], u4[:], MULT)
                v2 = pv.tile([P, FD], F32, tag="v")
                nc.scalar.activation(v2[:], m2[:], TANH, scale=p[14])

                cc = pa.tile([P, FD], F32, tag="a")
                nc.gpsimd.tensor_scalar(cc[:], v1[:], p[0], p[2], MULT, ADD)
                ot = po.tile([P, FD], F32, tag="o")
                nc.vector.scalar_tensor_tensor(ot[:], v2[:], p[1], cc[:], MULT, ADD)
                nc.sync.dma_start(out=o_h[:, sl], in_=ot[:])
    nc.compile()
    return nc


_cache = {}


def kernel(x, params):
    x = np.asarray(x)
    in_dtype = x.dtype
    x = np.ascontiguousarray(x, dtype=np.float32)
    params = np.asarray(params, dtype=np.float32)
    p = [float(v) for v in params]
    key = params.tobytes()
    if key not in _cache:
        _cache[key] = build_nc(p)
    nc = _cache[key]

    shards = x.reshape(NCORES, P, COLS)
    in_maps = [{"x": shards[i]} for i in range(NCORES)]
    trace = bool(int(os.environ.get("BASS_EXPR_TRACE", "0")))
    res = run_bass_kernel_spmd(nc, in_maps, list(range(NCORES)), trace=trace)
    out = np.concatenate(
        [res.results[i]["out"].reshape(-1) for i in range(NCORES)]
    ).astype(in_dtype, copy=False)
    if trace:
        kernel.last_exec_time_ns = res.exec_time_ns
        kernel.last_results = res
    return out


# revision 11
# speedup vs baseline: 1.0295x; 1.0295x over previous
"""Trainium2 Bass kernel for nn_ExpressionModule_2267742732789.

Expression tree (DEPTH=4, preorder params, 25 scalars), elementwise over x:
    x2 = x*x
    t1 = tanh(p7 *x2)   t2 = tanh(p8 *x2)   u1 = p4*t1 + p5*t2 + p6
    t3 = tanh(p12*x2)   t4 = tanh(p13*x2)   u2 = p9*t3 + p10*t4 + p11
    v1 = tanh(p3 * u1*u2)
    t5 = tanh(p18*x2)   t6 = tanh(p19*x2)   u3 = p15*t5 + p16*t6 + p17
    t7 = tanh(p23*x2)   t8 = tanh(p24*x2)   u4 = p20*t7 + p21*t8 + p22
    v2 = tanh(p14 * u3*u4)
    out = p0*v1 + p1*v2 + p2

Sharding: x (16M fp32) split evenly across the 8 NeuronCores (data
parallel, per the elementwise structure); the 25 scalar params are baked
into instruction immediates at call time (JIT specialization -- the kernel
recompiles for new param values, so it is correct for any input).

Engine split per 2M-element core shard (8 chunks of [128, 2048] fp32),
chosen by hardware A/B benchmarking (slope timing of K-pass kernels):
  ACT  (11 passes/elem): x^2 via Square + 8 leaf tanh (scale=p_k folded
        into the free pre-affine) + 2 mid tanh   <- binding engine
  DVE  (7 ops/chunk): 4 waff combines (scalar_tensor_tensor),
        2 tree products (tensor_tensor), final combine
  GPSIMD (5 ops/chunk): affine terms t*w + b (tensor_scalar)
  The x2 tiles live in PSUM (ACT PSUM-source ops save ~52 fixed cycles
  each, and the freed SBUF deepens the tanh-output pool to 7 bufs); the
  two mid tanhs are emitted after all leaf tanhs so the in-order ACT
  queue never stalls on DVE. Measured steady-state on TRN2: ~130 us/core
  (best-trial 113, spread to ~170 under tunnel/device load), at the
  11-pass ACT throughput floor (11 x 16384 elem/lane / 1.2 GHz = 150 us
  nominal). DMA (16.8 MB @ ~358 GB/s = 47 us) fully overlapped.

Findings that shaped this (from HW A/B runs):
  - DVE ops carry a ~0.5 us/op drain penalty on HW that the cost model
    misses -> minimizing DVE op count matters more than cycle balance.
  - Pool (GPSIMD) 2-input ops (tensor_tensor/STT) are ~2.6x slower than
    DVE and poison the critical path; only 1-input tensor_scalar is used.
  - ACT had slack at 10 passes (an extra pass was free), so x^2 moved
    there (-20 us); at 11 passes ACT binds (a 12th pass costs +29 us),
    and splitting x^2 back to DVE for 2-3 chunks is within noise.
  - In-place dataflow, ramp chunks, and larger FD all measured worse
    (ACT burst throttling / extra DVE drains / SBUF pressure).
"""

import os
import sys

import numpy as np

sys.path.insert(0, "/opt/trn_rl_repo")

import concourse.bacc as bacc
import concourse.mybir as mybir
from concourse import tile
from concourse.bass_utils import run_bass_kernel_spmd

N = 16777216
NCORES = 8
E = N // NCORES  # 2_097_152 per core
P = 128
COLS = E // P  # 16384 per-lane elements
FD = 2048
NCHUNK = COLS // FD  # 8

F32 = mybir.dt.float32
MULT = mybir.AluOpType.mult
ADD = mybir.AluOpType.add
TANH = mybir.ActivationFunctionType.Tanh
SQUARE = mybir.ActivationFunctionType.Square


def build_nc(p, passes=1):
    """Build the SPMD Bass program with params p (25 floats) baked in.

    passes>1 repeats the computation (same in/out) for benchmarking.
    """
    nc = bacc.Bacc("TRN2", target_bir_lowering=False, debug=False)
    x_h = nc.dram_tensor("x", [P, COLS], F32, kind="ExternalInput")
    o_h = nc.dram_tensor("out", [P, COLS], F32, kind="ExternalOutput")

    with tile.TileContext(nc) as tc:
        with (
            tc.tile_pool(name="px", bufs=3) as px,
            tc.tile_pool(name="po", bufs=3) as po,
            tc.tile_pool(name="px2", bufs=2, space="PSUM") as px2,
            tc.tile_pool(name="pt", bufs=7) as pt,
            tc.tile_pool(name="pa", bufs=3) as pa,
            tc.tile_pool(name="pu", bufs=3) as pu,
            tc.tile_pool(name="pm", bufs=2) as pm,
            tc.tile_pool(name="pv", bufs=2) as pv,
        ):
            for c in [c for _ in range(passes) for c in range(NCHUNK)]:
                sl = slice(c * FD, (c + 1) * FD)
                xt = px.tile([P, FD], F32, tag="x")
                nc.sync.dma_start(out=xt[:], in_=x_h[:, sl])
                x2 = px2.tile([P, FD], F32, tag="x2")
                nc.scalar.activation(x2[:], xt[:], SQUARE)

                def waff(s_a, s_b, w0, w1, b0):
                    ta = pt.tile([P, FD], F32, tag="t")
                    nc.scalar.activation(ta[:], x2[:], TANH, scale=s_a)
                    tb = pt.tile([P, FD], F32, tag="t")
                    nc.scalar.activation(tb[:], x2[:], TANH, scale=s_b)
                    aa = pa.tile([P, FD], F32, tag="a")
                    nc.gpsimd.tensor_scalar(aa[:], ta[:], w0, b0, MULT, ADD)
                    uu = pu.tile([P, FD], F32, tag="u")
                    nc.vector.scalar_tensor_tensor(uu[:], tb[:], w1, aa[:], MULT, ADD)
                    return uu

                u1 = waff(p[7], p[8], p[4], p[5], p[6])
                u2 = waff(p[12], p[13], p[9], p[10], p[11])
                m1 = pm.tile([P, FD], F32, tag="m")
                nc.vector.tensor_tensor(m1[:], u1[:], u2[:], MULT)
                u3 = waff(p[18], p[19], p[15], p[16], p[17])
                u4 = waff(p[23], p[24], p[20], p[21], p[22])
                m2 = pm.tile([P, FD], F32, tag="m")
                nc.vector.tensor_tensor(m2[:], u3[:], u4[:], MULT)
                # mid tanhs after all leaf tanhs: the in-order ACT queue
                # never waits on DVE mid-chunk
                v1 = pv.tile([P, FD], F32, tag="v")
                nc.scalar.activation(v1[:], m1[:], TANH, scale=p[3])
                v2 = pv.tile([P, FD], F32, tag="v")
                nc.scalar.activation(v2[:], m2[:], TANH, scale=p[14])

                cc = pa.tile([P, FD], F32, tag="a")
                nc.gpsimd.tensor_scalar(cc[:], v1[:], p[0], p[2], MULT, ADD)
                ot = po.tile([P, FD], F32, tag="o")
                nc.vector.scalar_tensor_tensor(ot[:], v2[:], p[1], cc[:], MULT, ADD)
                nc.sync.dma_start(out=o_h[:, sl], in_=ot[:])
    nc.compile()
    return nc


_cache = {}


def kernel(x, params):
    x = np.asarray(x)
    in_dtype = x.dtype
    x = np.ascontiguousarray(x, dtype=np.float32)
    params = np.asarray(params, dtype=np.float32)
    p = [float(v) for v in params]
    key = params.tobytes()
    if key not in _cache:
        _cache[key] = build_nc(p)
    nc = _cache[key]

    shards = x.reshape(NCORES, P, COLS)
    in_maps = [{"x": shards[i]} for i in range(NCORES)]
    trace = bool(int(os.environ.get("BASS_EXPR_TRACE", "0")))
    res = run_bass_kernel_spmd(nc, in_maps, list(range(NCORES)), trace=trace)
    out = np.concatenate(
        [res.results[i]["out"].reshape(-1) for i in range(NCORES)]
    ).astype(in_dtype, copy=False)
    if trace:
        kernel.last_exec_time_ns = res.exec_time_ns
        kernel.last_results = res
    return out


# revision 12
# speedup vs baseline: 1.4642x; 1.4223x over previous
"""Trainium2 Bass kernel for nn_ExpressionModule_2267742732789.

Expression tree (DEPTH=4, preorder params, 25 scalars), elementwise over x:
    x2 = x*x
    t1 = tanh(p7 *x2)   t2 = tanh(p8 *x2)   u1 = p4*t1 + p5*t2 + p6
    t3 = tanh(p12*x2)   t4 = tanh(p13*x2)   u2 = p9*t3 + p10*t4 + p11
    v1 = tanh(p3 * u1*u2)
    t5 = tanh(p18*x2)   t6 = tanh(p19*x2)   u3 = p15*t5 + p16*t6 + p17
    t7 = tanh(p23*x2)   t8 = tanh(p24*x2)   u4 = p20*t7 + p21*t8 + p22
    v2 = tanh(p14 * u3*u4)
    out = p0*v1 + p1*v2 + p2

Sharding: x (16M fp32) split evenly across the 8 NeuronCores (data
parallel, per the elementwise structure); the 25 scalar params are baked
into instruction immediates at call time (JIT specialization -- the kernel
recompiles for new param values, so it is correct for any input).

Engine split per 2M-element core shard (8 chunks of [128, 2048] fp32),
chosen by hardware A/B benchmarking (slope timing of K-pass kernels):
  ACT  (11 passes/elem): x^2 via Square + 8 leaf tanh (scale=p_k folded
        into the free pre-affine) + 2 mid tanh   <- binding engine
  DVE  (7 ops/chunk): 4 waff combines (scalar_tensor_tensor),
        2 tree products (tensor_tensor), final combine
  GPSIMD (5 ops/chunk): affine terms t*w + b (tensor_scalar)
  The x2 tiles live in PSUM (ACT PSUM-source ops save ~52 fixed cycles
  each, and the freed SBUF deepens the tanh-output pool to 7 bufs); the
  two mid tanhs are emitted after all leaf tanhs so the in-order ACT
  queue never stalls on DVE; m/v pools at 3 bufs let DVE products run
  ahead of the mid tanhs. Measured steady-state on TRN2: ~125 us/core
  (best-trial 105, spread to ~170 under tunnel/device load), at the
  11-pass ACT throughput floor (11 x 16384 elem/lane / 1.2 GHz = 150 us
  nominal). DMA (16.8 MB @ ~358 GB/s = 47 us) fully overlapped.

Findings that shaped this (from HW A/B runs):
  - DVE ops carry a ~0.5 us/op drain penalty on HW that the cost model
    misses -> minimizing DVE op count matters more than cycle balance.
  - Pool (GPSIMD) 2-input ops (tensor_tensor/STT) are ~2.6x slower than
    DVE and poison the critical path; only 1-input tensor_scalar is used.
  - ACT had slack at 10 passes (an extra pass was free), so x^2 moved
    there (-20 us); at 11 passes ACT binds (a 12th pass costs +29 us),
    and splitting x^2 back to DVE for 2-3 chunks is within noise.
  - In-place dataflow, ramp chunks, and larger FD all measured worse
    (ACT burst throttling / extra DVE drains / SBUF pressure).
"""

import os
import sys

import numpy as np

sys.path.insert(0, "/opt/trn_rl_repo")

import concourse.bacc as bacc
import concourse.mybir as mybir
from concourse import tile
from concourse.bass_utils import run_bass_kernel_spmd

N = 16777216
NCORES = 8
E = N // NCORES  # 2_097_152 per core
P = 128
COLS = E // P  # 16384 per-lane elements
FD = 2048
NCHUNK = COLS // FD  # 8

F32 = mybir.dt.float32
MULT = mybir.AluOpType.mult
ADD = mybir.AluOpType.add
TANH = mybir.ActivationFunctionType.Tanh
SQUARE = mybir.ActivationFunctionType.Square


def build_nc(p, passes=1):
    """Build the SPMD Bass program with params p (25 floats) baked in.

    passes>1 repeats the computation (same in/out) for benchmarking.
    """
    nc = bacc.Bacc("TRN2", target_bir_lowering=False, debug=False)
    x_h = nc.dram_tensor("x", [P, COLS], F32, kind="ExternalInput")
    o_h = nc.dram_tensor("out", [P, COLS], F32, kind="ExternalOutput")

    with tile.TileContext(nc) as tc:
        with (
            tc.tile_pool(name="px", bufs=3) as px,
            tc.tile_pool(name="po", bufs=3) as po,
            tc.tile_pool(name="px2", bufs=2, space="PSUM") as px2,
            tc.tile_pool(name="pt", bufs=7) as pt,
            tc.tile_pool(name="pa", bufs=3) as pa,
            tc.tile_pool(name="pu", bufs=3) as pu,
            tc.tile_pool(name="pm", bufs=3) as pm,
            tc.tile_pool(name="pv", bufs=3) as pv,
        ):
            for c in [c for _ in range(passes) for c in range(NCHUNK)]:
                sl = slice(c * FD, (c + 1) * FD)
                xt = px.tile([P, FD], F32, tag="x")
                nc.sync.dma_start(out=xt[:], in_=x_h[:, sl])
                x2 = px2.tile([P, FD], F32, tag="x2")
                nc.scalar.activation(x2[:], xt[:], SQUARE)

                def waff(s_a, s_b, w0, w1, b0):
                    ta = pt.tile([P, FD], F32, tag="t")
                    nc.scalar.activation(ta[:], x2[:], TANH, scale=s_a)
                    tb = pt.tile([P, FD], F32, tag="t")
                    nc.scalar.activation(tb[:], x2[:], TANH, scale=s_b)
                    aa = pa.tile([P, FD], F32, tag="a")
                    nc.gpsimd.tensor_scalar(aa[:], ta[:], w0, b0, MULT, ADD)
                    uu = pu.tile([P, FD], F32, tag="u")
                    nc.vector.scalar_tensor_tensor(uu[:], tb[:], w1, aa[:], MULT, ADD)
                    return uu

                u1 = waff(p[7], p[8], p[4], p[5], p[6])
                u2 = waff(p[12], p[13], p[9], p[10], p[11])
                m1 = pm.tile([P, FD], F32, tag="m")
                nc.vector.tensor_tensor(m1[:], u1[:], u2[:], MULT)
                u3 = waff(p[18], p[19], p[15], p[16], p[17])
                u4 = waff(p[23], p[24], p[20], p[21], p[22])
                m2 = pm.tile([P, FD], F32, tag="m")
                nc.vector.tensor_tensor(m2[:], u3[:], u4[:], MULT)
                # mid tanhs after all leaf tanhs: the in-order ACT queue
                # never waits on DVE mid-chunk
                v1 = pv.tile([P, FD], F32, tag="v")
                nc.scalar.activation(v1[:], m1[:], TANH, scale=p[3])
                v2 = pv.tile([P, FD], F32, tag="v")
                nc.scalar.activation(v2[:], m2[:], TANH, scale=p[14])

                cc = pa.tile([P, FD], F32, tag="a")
                nc.gpsimd.tensor_scalar(cc[:], v1[:], p[0], p[2], MULT, ADD)
                ot = po.tile([P, FD], F32, tag="o")
                nc.vector.scalar_tensor_tensor(ot[:], v2[:], p[1], cc[:], MULT, ADD)
                nc.sync.dma_start(out=o_h[:, sl], in_=ot[:])
    nc.compile()
    return nc


_cache = {}


def kernel(x, params):
    x = np.asarray(x)
    in_dtype = x.dtype
    x = np.ascontiguousarray(x, dtype=np.float32)
    params = np.asarray(params, dtype=np.float32)
    p = [float(v) for v in params]
    key = params.tobytes()
    if key not in _cache:
        _cache[key] = build_nc(p)
    nc = _cache[key]

    shards = x.reshape(NCORES, P, COLS)
    in_maps = [{"x": shards[i]} for i in range(NCORES)]
    trace = bool(int(os.environ.get("BASS_EXPR_TRACE", "0")))
    res = run_bass_kernel_spmd(nc, in_maps, list(range(NCORES)), trace=trace)
    out = np.concatenate(
        [res.results[i]["out"].reshape(-1) for i in range(NCORES)]
    ).astype(in_dtype, copy=False)
    if trace:
        kernel.last_exec_time_ns = res.exec_time_ns
        kernel.last_results = res
    return out
